# revision 28
# baseline (speedup 1.0000x reference)
"""Trainium2 Bass kernel for PVT-style spatial-reduction attention.

Reference computation (per batch):
  x_ds = x[:, ::4, ::4]                                  # nearest downsample 192->48
  q    = q_w @ x_ds + q_b                                # 1x1 conv
  d1   = relu(bn1(dwconv2x2_s2_p1(x_ds)))                # 48 -> 25
  kv1  = bn2(sr2_w @ d1)
  kv2  = dwconv3x3_s1_p1(kv1) + lc_b + kv1
  k,v  = split(kv_w @ kv2 + kv_b)
  out  = softmax(q'k/8) @ v  -> reshape [C,48,48] -> nearest upsample x4

Sharding: 8 cores = 4 batches x 2 head-groups (4 heads / 256 ch each).
Each core runs the identical Bass program on its (batch, head-group) shard
and writes its [256,192,192] slab of the output (stored fp16; host widens).

Schedule: the input load is split into 3 spatial blocks x 4 channel
chunks.  Per block, the whole kv pipeline (2x2 taps -> relu -> sr2 ->
3x3 taps -> k/v proj) and the q projection stream behind the DMA, so
attention can start right after the last block lands.  Attention+stores
then run with every engine pipelined; output is written fp16 which
halves the store traffic (error ~5e-4 rel, far under the 2e-2 gate).
"""

import sys

for _p in ("/root/.axon_site/_ro/trn_rl_repo", "/opt/trn_rl_repo"):
    if _p in sys.path:
        sys.path.remove(_p)
    sys.path.insert(0, _p)

import numpy as np
import ml_dtypes


def _ensure_ntff_hook_module():
    """Provide antenv.axon_hooks (NTFF profile hook registry) if the
    resolved antenv package lacks it — needed for trace=True profiling."""
    try:
        import antenv.axon_hooks  # noqa: F401

        return
    except ImportError:
        pass
    try:
        import types

        import antenv

        mod = types.ModuleType("antenv.axon_hooks")
        mod._HOOK = None

        def set_axon_ntff_profile_hook(hook):
            mod._HOOK = hook

        def get_axon_ntff_profile_hook():
            if mod._HOOK is None:
                try:
                    if "/root/.axon_site" not in sys.path:
                        sys.path.append("/root/.axon_site")
                    from trn_agent_boot.trn_boot import (
                        _ntff_profile_via_ctypes,
                    )

                    mod._HOOK = _ntff_profile_via_ctypes(
                        "/opt/axon/libaxon_pjrt.so"
                    )
                except Exception:
                    mod._HOOK = None
            return mod._HOOK

        mod.set_axon_ntff_profile_hook = set_axon_ntff_profile_hook
        mod.get_axon_ntff_profile_hook = get_axon_ntff_profile_hook
        antenv.axon_hooks = mod
        sys.modules["antenv.axon_hooks"] = mod
    except Exception:
        pass


_ensure_ntff_hook_module()

import concourse.bass as bass
import concourse.tile as tile
from concourse import bacc
from concourse import mybir
from concourse.bass_utils import run_bass_kernel_spmd

F32 = mybir.dt.float32
F32R = mybir.dt.float32r
F16 = mybir.dt.float16
BF16 = mybir.dt.bfloat16
ALU = mybir.AluOpType
ACTF = mybir.ActivationFunctionType

# Problem constants (hardcoded per contract).
C = 512          # channels
H0 = W0 = 192    # full spatial
HD = WD = 48     # downsampled spatial
N = HD * WD      # 2304 queries
HS = WS = 25     # spatially-reduced size after 2x2/s2/p1 dwconv
M = HS * WS      # 625 keys
HPC = 4          # heads per core
CQ = 256         # q/k/v channels per core
NCORES = 8
BN_EPS = 1e-5
SCALE = 0.125    # hd ** -0.5 = 64 ** -0.5

# 3 load blocks of 16 x_ds rows each.
XB = [(0, 16), (16, 16), (32, 16)]
# d1 rows completed per block
D1R = [(0, 8), (8, 8), (16, 9)]
# kv2 row-groups for the 3x3 taps / k / v projections (one per block).
KVG = [(0, 7), (7, 8), (15, 10)]
KVG_OF_BLOCK = [[0], [1], [2]]
# m-tiles over the 625 key positions (v output-partition tiles), aligned
# with the kv2 row-groups: tiles [0], [1,2], [3,4] complete after
# groups 0..2 respectively.
MTS = [(0, 128), (128, 128), (256, 119), (375, 125), (500, 125)]
VTILES = [[0], [1, 2], [3, 4]]
NSX = 6          # attention n-tiles per head pair (each 384 = 8 output rows)
NSZ = 384


def _build_nc():
    nc = bacc.Bacc("TRN2", target_bir_lowering=False, debug=False)

    xb = nc.dram_tensor("xb", [C, H0, W0], F32R, kind="ExternalInput").ap()
    qwT = nc.dram_tensor("qwT", [128, 4, CQ], F32R, kind="ExternalInput").ap()
    qb = nc.dram_tensor("qb", [128, 2], F32, kind="ExternalInput").ap()
    sr2T = nc.dram_tensor("sr2T", [128, 4, C], BF16, kind="ExternalInput").ap()
    kvkT = nc.dram_tensor("kvkT", [128, 4, CQ], BF16,
                          kind="ExternalInput").ap()
    kvvT = nc.dram_tensor("kvvT", [128, 4, CQ], BF16,
                          kind="ExternalInput").ap()
    kvbk = nc.dram_tensor("kvbk", [128, 2], F32, kind="ExternalInput").ap()
    kvbv = nc.dram_tensor("kvbv", [128, CQ], F32, kind="ExternalInput").ap()
    vecs = nc.dram_tensor("vecs", [128, 4, 18], F32, kind="ExternalInput").ap()
    out_d = nc.dram_tensor("out", [CQ, H0, W0], F16, kind="ExternalOutput").ap()

    with tile.TileContext(nc) as tc:
        with nc.allow_low_precision(
            reason="bf16/fp16 intermediates; matmul accumulation stays fp32"
        ):
            _body(tc, xb, qwT, qb, sr2T, kvkT, kvvT, kvbk, kvbv, vecs, out_d)
    nc.compile()
    return nc


def _body(tc, xb, qwT, qb, sr2T, kvkT, kvvT, kvbk, kvbv, vecs, out_d):
    nc = tc.nc
    from contextlib import ExitStack

    with ExitStack() as ctx:
        consts = ctx.enter_context(tc.tile_pool(name="consts", bufs=1))
        # small consts first so the x row DMAs start almost immediately;
        # the big weight tensors are issued between row-block DMAs, each
        # just ahead of its first use.
        vecs_sb = consts.tile([128, 4, 18], F32)
        nc.sync.dma_start(out=vecs_sb, in_=vecs)
        qb_sb = consts.tile([128, 2], F32)
        nc.sync.dma_start(out=qb_sb, in_=qb)
        kvbk_sb = consts.tile([128, 2], F32)
        nc.sync.dma_start(out=kvbk_sb, in_=kvbk)
        kvbv_sb = consts.tile([128, CQ], F32)
        nc.sync.dma_start(out=kvbv_sb, in_=kvbv)
        qwT_sb = consts.tile([128, 4, CQ], F32R)
        sr2T_sb = consts.tile([128, 4, C], BF16)
        kvkT_sb = consts.tile([128, 4, CQ], BF16)
        kvvT_sb = consts.tile([128, 4, CQ], BF16)

        zsmall = consts.tile([128, 1], F32)
        nc.vector.memset(zsmall, 0.0)
        osmall = consts.tile([128, 1], F32)
        nc.vector.memset(osmall, 1.0)
        ones1 = consts.tile([1, 64], BF16)
        nc.vector.tensor_copy(
            out=ones1, in_=osmall[0:1, :].to_broadcast([1, 64])
        )

        persist = ctx.enter_context(tc.tile_pool(name="persist", bufs=1))
        q_sb = persist.tile([128, 2, N], BF16)
        k_loc = persist.tile([128, 2, M], BF16)
        vT_sb = persist.tile([128, 5, HPC, 65], BF16)
        d1 = persist.tile([128, 4, M], F32R)       # tap accumulator (f32)
        d1b = persist.tile([128, 4, M], BF16)      # post-relu, sr2 rhs
        kv1 = persist.tile([128, 4, M], BF16)
        kv2 = persist.tile([128, 4, M], BF16)
        d1s = d1.rearrange("p c (h w) -> p c h w", h=HS)
        d1bs = d1b.rearrange("p c (h w) -> p c h w", h=HS)
        kv1s = kv1.rearrange("p c (h w) -> p c h w", h=HS)
        kv2s = kv2.rearrange("p c (h w) -> p c h w", h=HS)

        # zero d1 + set the vT ones column up front (hidden under consts DMA)
        nc.vector.tensor_copy(out=d1, in_=zsmall.to_broadcast([128, 4, M]))
        nc.vector.tensor_copy(
            out=vT_sb[:, :, :, 64], in_=osmall.to_broadcast([128, 5, HPC])
        )

        kvbv_h = kvbv_sb.rearrange("p (h d) -> p h d", h=HPC)
        xb_rows = xb.rearrange("c (h f) w -> c h f w", f=4)

        # ---- shared helpers (used by phase A and the phase-G tail) ----
        tmp_p = ctx.enter_context(tc.tile_pool(name="tmp", bufs=2))

        def taps3x3(g, gps_cc=()):
            # gps_cc chunks run on GPSIMD as mult+add pairs (used for the
            # last group, where DVE is the post-load critical path)
            kr0, krn = KVG[g]
            for cc in range(4):
                # center tap: kv2 = (w11 + 1) * kv1 + lc_b
                nc.scalar.activation(
                    out=kv2s[:, cc, kr0:kr0 + krn],
                    in_=kv1s[:, cc, kr0:kr0 + krn],
                    func=ACTF.Identity,
                    bias=vecs_sb[:, cc, 17:18],
                    scale=vecs_sb[:, cc, 12:13],
                )
                for ki in range(3):
                    for kj in range(3):
                        if ki == 1 and kj == 1:
                            continue
                        di, dj = ki - 1, kj - 1
                        a0 = max(kr0, max(0, -di))
                        a1 = min(kr0 + krn, 25 - max(0, di))
                        b0, b1 = max(0, -dj), 25 - max(0, dj)
                        src = kv1s[:, cc, a0 + di:a1 + di, b0 + dj:b1 + dj]
                        dst = kv2s[:, cc, a0:a1, b0:b1]
                        s = 8 + ki * 3 + kj
                        if cc in gps_cc:
                            nr, nco = a1 - a0, b1 - b0
                            w = vecs_sb[:, cc, s:s + 1]
                            tmp = tmp_p.tile([128, 10, 25], BF16, tag="tmp3")
                            tv = tmp[:, 0:nr, 0:nco]
                            nc.gpsimd.tensor_tensor(
                                out=tv, in0=src,
                                in1=w.to_broadcast([128, nr, nco]),
                                op=ALU.mult,
                            )
                            nc.gpsimd.tensor_tensor(
                                out=dst, in0=dst, in1=tv, op=ALU.add,
                            )
                        else:
                            nc.vector.scalar_tensor_tensor(
                                out=dst,
                                in0=src,
                                scalar=vecs_sb[:, cc, s:s + 1],
                                in1=dst,
                                op0=ALU.mult,
                                op1=ALU.add,
                            )

        def kproj(g, pool, fsz, tag):
            r0, rn = KVG[g]
            m0, mm = r0 * WS, rn * WS
            for kt in range(2):
                ps = pool.tile([128, fsz], F32, tag=tag)
                for cc in range(4):
                    nc.tensor.matmul(
                        ps[:, 0:mm],
                        lhsT=kvkT_sb[:, cc, kt * 128:(kt + 1) * 128],
                        rhs=kv2[:, cc, m0:m0 + mm],
                        start=(cc == 0),
                        stop=(cc == 3),
                    )
                nc.scalar.activation(
                    out=k_loc[:, kt, m0:m0 + mm],
                    in_=ps[:, 0:mm],
                    func=ACTF.Identity,
                    bias=kvbk_sb[:, kt:kt + 1],
                    scale=1.0,
                )

        def vproj(g, pool, fsz, tag):
            for mi in VTILES[g]:
                m0, msz = MTS[mi]
                ps = pool.tile([128, fsz], F32, tag=tag)
                for cc in range(4):
                    nc.tensor.matmul(
                        ps[:msz, 0:CQ],
                        lhsT=kv2[:, cc, m0:m0 + msz],
                        rhs=kvvT_sb[:, cc],
                        start=(cc == 0),
                        stop=(cc == 3),
                    )
                nc.vector.tensor_tensor(
                    out=vT_sb[:msz, mi, :, 0:64],
                    in0=ps[:msz, 0:CQ].rearrange("p (h d) -> p h d", h=HPC),
                    in1=kvbv_h[:msz],
                    op=ALU.add,
                )

        # ---- Phase A: blocked load + streamed kv pipeline + q-proj ----
        with ExitStack() as actx:
            rows_p = actx.enter_context(tc.tile_pool(name="rows", bufs=6))
            psQ = actx.enter_context(
                tc.tile_pool(name="psQ", bufs=3, space="PSUM")
            )
            psB = actx.enter_context(
                tc.tile_pool(name="psB", bufs=2, space="PSUM")
            )
            psV = actx.enter_context(
                tc.tile_pool(name="psV", bufs=2, space="PSUM")
            )

            nc.sync.dma_start(out=qwT_sb, in_=qwT)
            nc.sync.dma_start(out=sr2T_sb, in_=sr2T)
            for b in range(3):
                x0, xn = XB[b]
                rtiles = []
                for cc in range(4):
                    rt = rows_p.tile([128, 16, W0], F32R, tag="rows")
                    nc.sync.dma_start(
                        out=rt,
                        in_=xb_rows[cc * 128:(cc + 1) * 128, x0:x0 + xn, 0, :],
                    )
                    rtiles.append(rt)
                # remaining big consts staged just ahead of first use
                if b == 1:
                    nc.sync.dma_start(out=kvkT_sb, in_=kvkT)
                    nc.sync.dma_start(out=kvvT_sb, in_=kvvT)

                # 2x2/s2/p1 dwconv taps (GPSIMD mult+add pairs, keeping DVE
                # free for the 3x3 taps) + BN1+ReLU (ACT), per chunk.
                # d1 row r reads x_ds rows 2r-1 (ki=0), 2r (ki=1); block b
                # holds x_ds rows x0..x0+15, so ki=0 covers r in
                # [8b+1, 8b+8] (clipped to 24) and ki=1 covers [8b, 8b+7].
                d1r0, d1rn = D1R[b]
                for cc in range(4):
                    rv = rtiles[cc].rearrange(
                        "p (h2 two) (w8 e) -> p h2 two w8 e", two=2, e=8
                    )
                    for ki in (0, 1):
                        if ki == 0:
                            ra, rb_ = 8 * b + 1, min(8 * b + 8, 24)
                        else:
                            ra, rb_ = 8 * b, 8 * b + 7
                        nrow = rb_ - ra + 1
                        # local x row = 2r-1+ki - x0 -> h2 index, two=1-ki
                        h2a = (2 * ra - 1 + ki - x0) // 2
                        for kj in (0, 1):
                            ca, cb_ = (1, 24) if kj == 0 else (0, 23)
                            ncol = cb_ - ca + 1
                            e = 4 if kj == 0 else 0
                            src = rv[:, h2a:h2a + nrow, 1 - ki, 0:ncol, e]
                            dst = d1s[:, cc, ra:rb_ + 1, ca:cb_ + 1]
                            w = vecs_sb[:, cc, ki * 2 + kj:ki * 2 + kj + 1]
                            if b == 0:
                                nc.vector.scalar_tensor_tensor(
                                    out=dst, in0=src, scalar=w, in1=dst,
                                    op0=ALU.mult, op1=ALU.add,
                                )
                            else:
                                tmp = tmp_p.tile([128, 8, 24], F32, tag="tmp")
                                tv = tmp[:, 0:nrow, 0:ncol]
                                nc.gpsimd.tensor_tensor(
                                    out=tv, in0=src,
                                    in1=w.to_broadcast([128, nrow, ncol]),
                                    op=ALU.mult,
                                )
                                nc.gpsimd.tensor_tensor(
                                    out=dst, in0=dst, in1=tv, op=ALU.add,
                                )
                    # relu(s1*d1+b1) -> bf16 (rows d1r0..d1r0+d1rn-1 final)
                    nc.scalar.activation(
                        out=d1bs[:, cc, d1r0:d1r0 + d1rn],
                        in_=d1s[:, cc, d1r0:d1r0 + d1rn],
                        func=ACTF.Relu,
                        bias=vecs_sb[:, cc, 5:6],
                        scale=vecs_sb[:, cc, 4:5],
                    )

                # q projection for this block's 768 queries (2 n-tiles of 384)
                for mt in range(2):
                    for ntl in range(2):
                        ps = psQ.tile([128, NSZ], F32, tag="psQ")
                        for cc in range(4):
                            rq = rtiles[cc].rearrange(
                                "p h (w f) -> p h w f", f=4
                            )
                            nc.tensor.matmul(
                                ps,
                                lhsT=qwT_sb[:, cc, mt * 128:(mt + 1) * 128],
                                rhs=rq[:, ntl * 8:(ntl + 1) * 8, :, 0],
                                start=(cc == 0),
                                stop=(cc == 3),
                            )
                        nt0 = (2 * b + ntl) * NSZ
                        nc.scalar.activation(
                            out=q_sb[:, mt, nt0:nt0 + NSZ],
                            in_=ps,
                            func=ACTF.Identity,
                            bias=qb_sb[:, mt:mt + 1],
                            scale=1.0,
                        )

                # sr2 1x1 conv + BN2 for this block's d1 rows
                m0, mm = d1r0 * WS, d1rn * WS
                for mt in range(4):
                    ps = psB.tile([128, 512], F32, tag="psB")
                    for cc in range(4):
                        nc.tensor.matmul(
                            ps[:, 0:mm],
                            lhsT=sr2T_sb[:, cc, mt * 128:(mt + 1) * 128],
                            rhs=d1b[:, cc, m0:m0 + mm],
                            start=(cc == 0),
                            stop=(cc == 3),
                        )
                    nc.scalar.activation(
                        out=kv1[:, mt, m0:m0 + mm],
                        in_=ps[:, 0:mm],
                        func=ACTF.Identity,
                        bias=vecs_sb[:, mt, 7:8],
                        scale=vecs_sb[:, mt, 6:7],
                    )

                # k/v projections for the PREVIOUS block's kv2 row-group
                # (its 3x3 taps had a full block window to finish), then
                # this block's 3x3 taps (except the last group's, which are
                # emitted inside phase G so attention pre-runs first).
                if b > 0:
                    kproj(b - 1, psB, 512, "psB")
                    vproj(b - 1, psV, CQ, "psV")
                if b < 2:
                    taps3x3(b)

        # ---- Phase G: attention, normalize, upsample, store (fp16) ----
        # While the last group's 3x3 taps drain on DVE/GPSIMD, the first
        # attention units pre-run their qk/exp/oav for the already-complete
        # m-tiles 0..2, keeping PE busy through the tail.
        PRE = [(0, 0), (0, 1)]
        with ExitStack() as gctx:
            qk_pool = gctx.enter_context(
                tc.tile_pool(name="qk", bufs=3, space="PSUM")
            )
            oa_pool = gctx.enter_context(
                tc.tile_pool(name="oa", bufs=4, space="PSUM")
            )
            bc_pool = gctx.enter_context(
                tc.tile_pool(name="bc", bufs=1, space="PSUM")
            )
            e_pool = gctx.enter_context(tc.tile_pool(name="es", bufs=6))
            uw_pool = gctx.enter_context(tc.tile_pool(name="uw", bufs=3))
            r_pool = gctx.enter_context(tc.tile_pool(name="rp", bufs=3))

            def head_mis(pr, t6, hh, oa, mis):
                h = 2 * pr + hh
                hp = hh * 64
                t0 = t6 * NSZ
                qks = []
                for mi in mis:
                    m0, msz = MTS[mi]
                    ps = qk_pool.tile([128, NSZ], F32, tag="qk")
                    nc.tensor.matmul(
                        ps[:msz],
                        lhsT=k_loc[hp:hp + 64, pr, m0:m0 + msz],
                        rhs=q_sb[hp:hp + 64, pr, t0:t0 + NSZ],
                        start=True,
                        stop=True,
                    )
                    qks.append(ps)
                es = []
                for mi, ps in zip(mis, qks):
                    msz = MTS[mi][1]
                    e = e_pool.tile([128, NSZ], BF16, tag="es")
                    nc.scalar.activation(
                        out=e[:msz],
                        in_=ps[:msz],
                        func=ACTF.Exp,
                        scale=SCALE,
                    )
                    es.append(e)
                for mi, e in zip(mis, es):
                    msz = MTS[mi][1]
                    nc.tensor.matmul(
                        oa,
                        lhsT=vT_sb[:msz, mi, h, :],
                        rhs=e[:msz],
                        start=(mi == 0),
                        stop=(mi == 4),
                    )

            # last group's 3x3 taps (DVE chunks 0-2, GPSIMD chunk 3),
            # then the pre-run units, then the last k/v projections.
            taps3x3(2, gps_cc=(3,))
            pre = {}
            for (pr, t6) in PRE:
                for hh in range(2):
                    oa = oa_pool.tile([65, NSZ], F32, tag="oa")
                    head_mis(pr, t6, hh, oa, [0, 1, 2])
                    pre[(pr, t6, hh)] = oa
            kproj(2, qk_pool, NSZ, "qk")
            vproj(2, qk_pool, NSZ, "qk")

            for pr in range(2):
                dst4 = out_d[pr * 128:(pr + 1) * 128].rearrange(
                    "c (t a fp two) w -> c t a fp (two w)", t=NSX, fp=2, two=2
                )
                for t6 in range(NSX):
                    oas = []
                    for hh in range(2):
                        if (pr, t6, hh) in pre:
                            oa = pre.pop((pr, t6, hh))
                            head_mis(pr, t6, hh, oa, [3, 4])
                        else:
                            oa = oa_pool.tile([65, NSZ], F32, tag="oa")
                            head_mis(pr, t6, hh, oa, [0, 1, 2, 3, 4])
                        oas.append(oa)

                    # epilogue: stage both denominators on one partition,
                    # broadcast via tiny matmuls, one 128-partition
                    # reciprocal, normalize+col-replicate + row-replicate
                    # (DVE), store
                    rt2 = r_pool.tile([1, 2, NSZ], BF16, tag="rt2")
                    for hh in range(2):
                        nc.vector.tensor_copy(
                            out=rt2[0:1, hh], in_=oas[hh][64:65]
                        )
                    bc = bc_pool.tile([128, NSZ], F32, tag="bc")
                    for hh in range(2):
                        nc.tensor.matmul(
                            bc[hh * 64:(hh + 1) * 64],
                            lhsT=ones1,
                            rhs=rt2[0:1, hh],
                            start=True,
                            stop=True,
                        )
                    bcs = r_pool.tile([128, NSZ], F32, tag="bcs")
                    nc.vector.reciprocal_approx_fast(out=bcs, in_=bc)
                    bcv = bcs.rearrange("p (a w) -> p a w", w=48)

                    uw = uw_pool.tile([128, 8, 2, W0], F16, tag="uw")
                    uw4 = uw.rearrange("p a two (w f) -> p a two w f", f=4)
                    for hh in range(2):
                        pb = hh * 64
                        src0 = oas[hh][0:64].rearrange(
                            "p (a w) -> p a w", w=48
                        ).unsqueeze(3).broadcast_to([64, 8, 48, 4])
                        src1 = bcv[pb:pb + 64].unsqueeze(3).broadcast_to(
                            [64, 8, 48, 4]
                        )
                        nc.vector.tensor_tensor(
                            out=uw4[pb:pb + 64, :, 0, :, :],
                            in0=src0,
                            in1=src1,
                            op=ALU.mult,
                        )
                    nc.vector.tensor_copy(
                        out=uw[:, :, 1, :], in_=uw[:, :, 0, :]
                    )

                    uw_f = uw.rearrange("p a two w -> p a (two w)")
                    for fp in range(2):
                        nc.sync.dma_start(
                            out=dst4[:, t6, :, fp, :], in_=uw_f
                        )


_NC_CACHE = None


def _get_nc():
    global _NC_CACHE
    if _NC_CACHE is None:
        _NC_CACHE = _build_nc()
    return _NC_CACHE


def _prep_in_maps(inputs):
    x = np.ascontiguousarray(np.asarray(inputs["x"], dtype=np.float32))
    q_w = np.asarray(inputs["q_w"], dtype=np.float32)
    q_b = np.asarray(inputs["q_b"], dtype=np.float32)
    kv_w = np.asarray(inputs["kv_w"], dtype=np.float32)
    kv_b = np.asarray(inputs["kv_b"], dtype=np.float32)
    sr1_w = np.asarray(inputs["sr1_w"], dtype=np.float32)
    bn1 = [np.asarray(inputs[f"bn1_{t}"], dtype=np.float32) for t in "gbmv"]
    sr2_w = np.asarray(inputs["sr2_w"], dtype=np.float32)
    bn2 = [np.asarray(inputs[f"bn2_{t}"], dtype=np.float32) for t in "gbmv"]
    lc_w = np.asarray(inputs["lc_w"], dtype=np.float32)
    lc_b = np.asarray(inputs["lc_b"], dtype=np.float32)

    def chan_layout(vec_2d, dtype=np.float32):
        # [C, k] -> [128, 4, k] with channel = cc*128 + p
        k = vec_2d.shape[1]
        return np.ascontiguousarray(
            vec_2d.reshape(4, 128, k).transpose(1, 0, 2).astype(dtype)
        )

    s1 = bn1[0] / np.sqrt(bn1[3] + BN_EPS)
    b1 = bn1[1] - bn1[2] * s1
    s2 = bn2[0] / np.sqrt(bn2[3] + BN_EPS)
    b2 = bn2[1] - bn2[2] * s2
    lc = lc_w.reshape(C, 9).copy()
    lc[:, 4] += 1.0  # fold residual into center tap
    vecs = np.zeros((C, 18), np.float32)
    vecs[:, 0:4] = sr1_w.reshape(C, 4)
    vecs[:, 4] = s1
    vecs[:, 5] = b1
    vecs[:, 6] = s2
    vecs[:, 7] = b2
    vecs[:, 8:17] = lc
    vecs[:, 17] = lc_b
    vecs_l = chan_layout(vecs)
    bf16 = ml_dtypes.bfloat16
    sr2T_l = chan_layout(sr2_w.T.copy(), bf16)  # [in-ch, out-ch]

    in_maps = []
    for b in range(4):
        for hg in range(2):
            sl = slice(hg * CQ, (hg + 1) * CQ)
            qwT = chan_layout(q_w[sl].T.copy())
            qb_l = np.ascontiguousarray(q_b[sl].reshape(2, 128).T)
            kvkT = chan_layout(kv_w[sl].T.copy(), bf16)
            kvvT = chan_layout(
                kv_w[C + hg * CQ:C + (hg + 1) * CQ].T.copy(), bf16
            )
            kvbk = np.ascontiguousarray(kv_b[sl].reshape(2, 128).T)
            kvbv = np.ascontiguousarray(
                np.broadcast_to(
                    kv_b[C + hg * CQ:C + (hg + 1) * CQ], (128, CQ)
                ).copy()
            )
            in_maps.append(
                {
                    "xb": x[b],
                    "qwT": qwT,
                    "qb": qb_l,
                    "sr2T": sr2T_l,
                    "kvkT": kvkT,
                    "kvvT": kvvT,
                    "kvbk": kvbk,
                    "kvbv": kvbv,
                    "vecs": vecs_l,
                }
            )
    return in_maps


def run(inputs, trace=False, **spmd_kwargs):
    """Run the SPMD kernel; returns (output, BassKernelResults)."""
    nc = _get_nc()
    in_maps = _prep_in_maps(inputs)
    res = run_bass_kernel_spmd(
        nc, in_maps, core_ids=list(range(NCORES)), trace=trace, **spmd_kwargs
    )
    out = np.empty((4, C, H0, W0), np.float32)
    i = 0
    for b in range(4):
        for hg in range(2):
            out[b, hg * CQ:(hg + 1) * CQ] = res.results[i]["out"]
            i += 1
    return out, res


def kernel(**inputs):
    out, _ = run(inputs, trace=False)
    return out
